# Initial kernel scaffold
#
"""Causal self-attention (B=4, S=2048, H=1024, NH=16) on 8 Trainium2 cores.

Sharding: core c -> (batch b = c//2, head-group g = c%2). Each core computes
8 heads (512 model dims) for one batch: QKV projections, causal attention,
and a partial output projection. Host sums the two head-group partials per
batch and adds bo.

Per-core kernel layout choices:
  - QT/KT computed in (dq, s) layout (head dims on partitions) so the scores
    matmul produces scores^T tiles (s_k on partitions, s_q free). Softmax
    runs without max-subtraction (inputs are well-scaled); exp on ACT,
    causal mask via gpsimd affine_select on the exp output, denominator via
    a ones-column matmul (col-packed with per-head PV matmuls), and the
    1/denom normalization is broadcast across partitions with a K=1 matmul.
  - V in natural (s, dv) layout feeds PV matmuls as the stationary operand;
    output lands pre-transposed (dv, s) = exactly the lhsT layout the output
    projection needs.
  - Matmuls run as float32r (full-rate fp32 streaming); P and V are bf16.
"""

import numpy as np

import concourse.bass as bass
import concourse.mybir as mybir
from concourse import bacc
from concourse.tile import TileContext
from concourse.bass_utils import run_bass_kernel_spmd

F32 = mybir.dt.float32
F32R = mybir.dt.float32r
BF16 = mybir.dt.bfloat16

B, S, H = 4, 2048, 1024
NH, HD = 16, 64
P = 128
DH = 512          # model dims per core (8 heads)
NHP = 4           # head pairs per core
SQC = 512         # s_q chunk (free dim of score tiles)
NSQ = S // SQC    # 4
NSK = S // P      # 16 s_k chunks
HO = H // P       # 8 contraction chunks for projections
NDQ = DH // P     # 4 dq tiles
SCALE = 0.125     # 1/sqrt(HD)


def r(ap):
    return ap.bitcast(F32R)


def build_kernel() -> bacc.Bacc:
    nc = bacc.Bacc("TRN2", target_bir_lowering=False, debug=False, num_devices=8)

    xT = nc.dram_tensor("xT", [H, S], F32, kind="ExternalInput").ap()
    wqT = nc.dram_tensor("wqT", [H, DH], F32, kind="ExternalInput").ap()
    wkT = nc.dram_tensor("wkT", [H, DH], F32, kind="ExternalInput").ap()
    wvT = nc.dram_tensor("wvT", [H, DH], F32, kind="ExternalInput").ap()
    woT = nc.dram_tensor("woT", [DH, H], F32, kind="ExternalInput").ap()
    bq = nc.dram_tensor("bq", [DH], F32, kind="ExternalInput").ap()
    bk = nc.dram_tensor("bk", [DH], F32, kind="ExternalInput").ap()
    bv = nc.dram_tensor("bv", [DH], F32, kind="ExternalInput").ap()
    out = nc.dram_tensor("out", [S, H], F32, kind="ExternalOutput").ap()

    with TileContext(nc) as tc:
        consts = tc.alloc_tile_pool(name="consts", bufs=1)
        persist = tc.alloc_tile_pool(name="persist", bufs=1)
        dram = tc.alloc_tile_pool(name="dram", bufs=1, space="DRAM")

        ones_sb = consts.tile([P, P], F32)       # all-ones; rows used as K=1 lhsT
        nc.any.memset(ones_sb, 1.0)
        ind_sb = consts.tile([P, P], F32)        # row 0: head-A cols; row 32: head-B
        nc.any.memset(ind_sb, 0.0)
        nc.any.memset(ind_sb[0:1, 0:64], 1.0)
        nc.any.memset(ind_sb[32:33, 64:128], 1.0)
        ones_col = consts.tile([P, 1], BF16)     # denominator lhsT
        nc.any.memset(ones_col, 1.0)
        bq_sb = consts.tile([P, NDQ], F32)
        nc.sync.dma_start(bq_sb[:], bq.rearrange("(o p) -> p o", p=P))
        bk_sb = consts.tile([P, NDQ], F32)
        nc.sync.dma_start(bk_sb[:], bk.rearrange("(o p) -> p o", p=P))
        bv_sb = consts.tile([1, DH], F32)
        nc.sync.dma_start(bv_sb[:], bv[None, :])

        v_sb = persist.tile([P, NSK, DH], BF16)      # V (s, dv), bf16
        outT_sb = persist.tile([P, NDQ, S], F32)     # normalized attn out^T
        qt_dram = dram.tile([NDQ, P, S], F32)        # Q^T spill
        kt_dram = dram.tile([NDQ, P, S], F32)        # K^T spill

        # ---- Phase 1: projections ----------------------------------------
        with (
            tc.tile_pool(name="p1_sbuf", bufs=1) as p1,
            tc.tile_pool(name="p1_stage", bufs=6) as stage,
            tc.tile_pool(name="p1_psum", bufs=4, space="PSUM") as pp,
        ):
            xT_sb = p1.tile([P, HO, S], F32)
            for o in range(HO):
                nc.sync.dma_start(
                    xT_sb[:, o, :], xT.rearrange("(o p) s -> p o s", p=P)[:, o, :]
                )
            w_sb = {}
            for name, wT in (("q", wqT), ("k", wkT), ("v", wvT)):
                w_sb[name] = p1.tile([P, HO, DH], F32, name=f"w{name}_sb")
                for o in range(HO):
                    nc.sync.dma_start(
                        w_sb[name][:, o, :],
                        wT.rearrange("(o p) d -> p o d", p=P)[:, o, :],
                    )

            # Q^T and K^T: psum[dq 128, s 512] = sum_o w[o, dq].T @ xT[o, s]
            for name, bias_sb, scale, tgt in (
                ("q", bq_sb, SCALE, qt_dram),
                ("k", bk_sb, 1.0, kt_dram),
            ):
                for t in range(NDQ):
                    for sc in range(NSQ):
                        ps = pp.tile([P, SQC], F32, name="proj_ps")
                        for o in range(HO):
                            nc.tensor.matmul(
                                ps[:],
                                r(w_sb[name][:, o, t * P : (t + 1) * P]),
                                r(xT_sb[:, o, sc * SQC : (sc + 1) * SQC]),
                                start=(o == 0),
                                stop=(o == HO - 1),
                            )
                        st = stage.tile([P, SQC], F32, name="qk_stage")
                        nc.scalar.activation(
                            st[:], ps[:],
                            mybir.ActivationFunctionType.Identity,
                            bias=bias_sb[:, t : t + 1], scale=scale,
                        )
                        nc.sync.dma_start(tgt[t, :, sc * SQC : (sc + 1) * SQC], st[:])

            # V: psum[s 128, dv 512] = sum_o xT[o, s].T @ wv[o, dv]  (+ bv)
            for st_i in range(NSK):
                ps = pp.tile([P, DH], F32, name="v_ps")
                for o in range(HO):
                    nc.tensor.matmul(
                        ps[:],
                        r(xT_sb[:, o, st_i * P : (st_i + 1) * P]),
                        r(w_sb["v"][:, o, :]),
                        start=(o == 0),
                        stop=False,
                    )
                nc.tensor.matmul(  # bias row: ones[1,128].T @ bv[1,512]
                    ps[:], r(ones_sb[0:1, :]), r(bv_sb[0:1, :]),
                    start=False, stop=True,
                )
                nc.vector.tensor_copy(v_sb[:, st_i, :], ps[:])

        # ---- Phase 2: attention ------------------------------------------
        with (
            tc.tile_pool(name="p2_wo", bufs=1) as p2w,
            tc.tile_pool(name="p2_qk", bufs=2) as p2qk,
            tc.tile_pool(name="p2_p", bufs=36) as p2p,
            tc.tile_pool(name="p2_rc", bufs=2) as p2rc,
            tc.tile_pool(name="ps_s", bufs=4, space="PSUM") as ps_s,
            tc.tile_pool(name="ps_pv", bufs=2, space="PSUM") as ps_pv,
            tc.tile_pool(name="ps_den", bufs=2, space="PSUM") as ps_den,
            tc.tile_pool(name="ps_bc", bufs=1, space="PSUM") as ps_bc,
        ):
            wo_sb = p2w.tile([P, NDQ, H], F32)
            for o in range(NDQ):
                nc.sync.dma_start(
                    wo_sb[:, o, :], woT.rearrange("(o p) m -> p o m", p=P)[:, o, :]
                )

            pending = None  # deferred normalize: (pv_ps, den_ps, hp, i)
            for hp in range(NHP):
                qt_hp = p2qk.tile([P, S], F32, name="qt_hp")
                nc.sync.dma_start(qt_hp[:], qt_dram[hp])
                kt_hp = p2qk.tile([P, S], F32, name="kt_hp")
                nc.sync.dma_start(kt_hp[:], kt_dram[hp])

                for i in range(NSQ):
                    nj = 4 * i + 4  # causal s_k chunks
                    sq = slice(i * SQC, (i + 1) * SQC)
                    p_tiles = []
                    for j in range(nj):
                        sk = slice(j * P, (j + 1) * P)
                        pj = []
                        for h, (pb, tp) in enumerate(((0, (0, 0)), (64, (64, 0)))):
                            sc_ps = ps_s.tile([P, SQC], F32, name="sc_ps")
                            nc.tensor.matmul(
                                sc_ps[:],
                                r(kt_hp[pb : pb + 64, sk]),
                                r(qt_hp[pb : pb + 64, sq]),
                                start=True, stop=True,
                                tile_position=tp,
                            )
                            pt = p2p.tile([P, SQC], BF16, name="p_tile")
                            nc.scalar.activation(
                                pt[:], sc_ps[:], mybir.ActivationFunctionType.Exp
                            )
                            if j >= 4 * i:  # diagonal block: causal mask
                                nc.gpsimd.affine_select(
                                    pt[:], pt[:],
                                    pattern=[[1, SQC]],
                                    compare_op=mybir.AluOpType.is_ge,
                                    fill=0.0,
                                    base=SQC * i - P * j,
                                    channel_multiplier=-1,
                                )
                            pj.append(pt)
                        p_tiles.append(pj)

                    den_ps = ps_den.tile([P, SQC], F32, name="den_ps")
                    pv_ps = ps_pv.tile([P, SQC], F32, name="pv_ps")
                    for j in range(nj):
                        st, sp = (j == 0), (j == nj - 1)
                        for h, (rowbase, colpos) in enumerate(((0, 0), (32, 32))):
                            nc.tensor.matmul(
                                den_ps[rowbase : rowbase + 1, :],
                                ones_col[:, 0:1],
                                p_tiles[j][h][:],
                                start=st, stop=sp,
                                tile_position=(0, colpos),
                            )
                        for h in range(2):
                            dv = slice(hp * P + h * 64, hp * P + h * 64 + 64)
                            nc.tensor.matmul(
                                pv_ps[h * 64 : h * 64 + 64, :],
                                v_sb[:, j, dv],
                                p_tiles[j][h][:],
                                start=st, stop=sp,
                                tile_position=(0, h * 64),
                            )

                    rc = p2rc.tile([P, SQC], F32, name="rc")
                    nc.vector.reciprocal(rc[0:1, :], den_ps[0:1, :])
                    nc.vector.reciprocal(rc[32:33, :], den_ps[32:33, :])

                    if pending is not None:
                        _flush_norm(nc, ps_bc, ind_sb, *pending)
                    pending = (pv_ps, rc, hp, i)
            _flush_norm(nc, ps_bc, ind_sb, *pending)

            # ---- Phase 3: output projection ------------------------------
            with (
                tc.tile_pool(name="p3_stage", bufs=4) as p3s,
                tc.tile_pool(name="ps_o", bufs=4, space="PSUM") as ps_o,
            ):
                for st_i in range(NSK):
                    ss = slice(st_i * P, (st_i + 1) * P)
                    for mc in range(2):
                        ms = slice(mc * SQC, (mc + 1) * SQC)
                        ps = ps_o.tile([P, SQC], F32, name="o_ps")
                        for ko in range(NDQ):
                            nc.tensor.matmul(
                                ps[:],
                                r(outT_sb[:, ko, ss]),
                                r(wo_sb[:, ko, ms]),
                                start=(ko == 0), stop=(ko == NDQ - 1),
                            )
                        ot = p3s.tile([P, SQC], F32, name="o_stage")
                        nc.vector.tensor_copy(ot[:], ps[:])
                        nc.sync.dma_start(out[ss, ms], ot[:])

    nc.compile()
    return nc


def _flush_norm(nc, ps_bc, ind_sb, pv_ps, rc, hp, i):
    """outT[:, hp, sq(i)] = pv_ps * broadcast(1/denom) via K=1 matmuls."""
    bc = ps_bc.tile([P, SQC], F32, name="bc_ps")
    nc.tensor.matmul(
        bc[:], r(ind_sb[0:1, :]), r(rc[0:1, :]),
        start=True, stop=False, tile_position=(0, 0),
    )
    nc.tensor.matmul(
        bc[:], r(ind_sb[32:33, :]), r(rc[32:33, :]),
        start=False, stop=True, tile_position=(32, 0),
    )
    outT_sb = _OUTT[0]
    nc.vector.tensor_mul(
        outT_sb[:, hp, i * SQC : (i + 1) * SQC], pv_ps[:], bc[:]
    )


_OUTT = [None]
_NC_CACHE = [None]


def kernel(x, Wq, bq, Wk, bk, Wv, bv, Wo, bo):
    x, Wq, bq, Wk, bk, Wv, bv, Wo, bo = (
        np.asarray(a, dtype=np.float32) for a in (x, Wq, bq, Wk, bk, Wv, bv, Wo, bo)
    )
    if _NC_CACHE[0] is None:
        _NC_CACHE[0] = build_kernel()
    nc = _NC_CACHE[0]

    in_maps = []
    for c in range(8):
        b, g = c // 2, c % 2
        hs = slice(DH * g, DH * (g + 1))
        in_maps.append({
            "xT": np.ascontiguousarray(x[b].T),
            "wqT": np.ascontiguousarray(Wq[hs].T),
            "wkT": np.ascontiguousarray(Wk[hs].T),
            "wvT": np.ascontiguousarray(Wv[hs].T),
            "woT": np.ascontiguousarray(Wo[:, hs].T),
            "bq": np.ascontiguousarray(bq[hs]) * np.float32(SCALE),
            "bk": np.ascontiguousarray(bk[hs]),
            "bv": np.ascontiguousarray(bv[hs]),
        })
    res = run_bass_kernel_spmd(nc, in_maps, core_ids=list(range(8)))
    out = np.empty((B, S, H), np.float32)
    for b in range(B):
        out[b] = res.results[2 * b]["out"] + res.results[2 * b + 1]["out"] + bo
    return out


# revision 10
# speedup vs baseline: 1.4297x; 1.4297x over previous
"""Causal self-attention (B=4, S=2048, H=1024, NH=16) on 8 Trainium2 cores.

Sharding: core c -> (batch b = c//2, head-group g = c%2). Each core computes
8 heads (512 model dims) for one batch: QKV projections, causal attention,
and a partial output projection. Host sums the two head-group partials per
batch and adds bo.

Per-core kernel layout choices:
  - QT/KT computed in (dq, s) layout (head dims on partitions) so the scores
    matmul produces scores^T tiles (s_k on partitions, s_q free). Softmax
    runs without max-subtraction (inputs are well-scaled); exp on ACT,
    causal mask via gpsimd affine_select on the exp output, denominator via
    a ones-column matmul (col-packed with per-head PV matmuls), and the
    1/denom normalization is broadcast across partitions with a K=1 matmul.
  - V in natural (s, dv) layout feeds PV matmuls as the stationary operand;
    output lands pre-transposed (dv, s) = exactly the lhsT layout the output
    projection needs.
  - Matmuls run as float32r (full-rate fp32 streaming); P and V are bf16.
"""

import os
from contextlib import ExitStack

import numpy as np

import concourse.bass as bass
import concourse.mybir as mybir
from concourse import bacc
from concourse.tile import TileContext
from concourse.bass_utils import run_bass_kernel_spmd

F32 = mybir.dt.float32
F32R = mybir.dt.float32r
BF16 = mybir.dt.bfloat16

B, S, H = 4, 2048, 1024
NH, HD = 16, 64
P = 128
DH = 512          # model dims per core (8 heads)
NHP = 4           # head pairs per core
SQC = 512         # s_q chunk (free dim of score tiles)
NSQ = S // SQC    # 4
NSK = S // P      # 16 s_k chunks
HO = H // P       # 8 contraction chunks for projections
NDQ = DH // P     # 4 dq tiles
SCALE = 0.125     # 1/sqrt(HD)


def r(ap):
    return ap.bitcast(F32R)


def build_kernel() -> bacc.Bacc:
    nc = bacc.Bacc("TRN2", target_bir_lowering=False, debug=False, num_devices=8)

    xT = nc.dram_tensor("xT", [H, S], F32, kind="ExternalInput").ap()
    wqT = nc.dram_tensor("wqT", [H, DH], F32, kind="ExternalInput").ap()
    wkT = nc.dram_tensor("wkT", [H, DH], F32, kind="ExternalInput").ap()
    wvT = nc.dram_tensor("wvT", [H, DH], F32, kind="ExternalInput").ap()
    woT = nc.dram_tensor("woT", [DH, H], F32, kind="ExternalInput").ap()
    bq = nc.dram_tensor("bq", [DH], F32, kind="ExternalInput").ap()
    bk = nc.dram_tensor("bk", [DH], F32, kind="ExternalInput").ap()
    bv = nc.dram_tensor("bv", [DH], F32, kind="ExternalInput").ap()
    cmat = nc.dram_tensor("cmat", [P, P], F32, kind="ExternalInput").ap()
    out = nc.dram_tensor("out", [S, H], F32, kind="ExternalOutput").ap()

    with (
        TileContext(nc) as tc,
        ExitStack() as ctx,
        nc.allow_low_precision(reason="float32r is bit-identical to float32"),
    ):
        consts = ctx.enter_context(tc.tile_pool(name="consts", bufs=1))
        persist = ctx.enter_context(tc.tile_pool(name="persist", bufs=1))
        dram = ctx.enter_context(tc.tile_pool(name="dram", bufs=1, space="DRAM"))

        # cmat rows: 0 = head-A indicator, 32 = head-B indicator, 64 = ones
        ind_sb = consts.tile([P, P], F32R)
        nc.sync.dma_start(ind_sb[:], cmat.bitcast(F32R))
        ones_col = consts.tile([P, 1], BF16)     # denominator lhsT
        nc.any.memset(ones_col, 1.0)
        bq_sb = consts.tile([P, NDQ], F32)
        nc.sync.dma_start(bq_sb[:], bq.rearrange("(o p) -> p o", p=P))
        bk_sb = consts.tile([P, NDQ], F32)
        nc.sync.dma_start(bk_sb[:], bk.rearrange("(o p) -> p o", p=P))
        bv_sb = consts.tile([P, DH], F32R)
        nc.sync.dma_start(bv_sb[64:65, :], bv[None, :].bitcast(F32R))

        v_sb = persist.tile([P, NSK, DH], BF16)      # V (s, dv), bf16
        outT_sb = persist.tile([P, NDQ, S], F32R)    # normalized attn out^T
        qt_dram = dram.tile([NDQ, P, S], F32R)       # Q^T spill
        kt_dram = dram.tile([NDQ, P, S], F32R)       # K^T spill

        # ---- Phase 1: projections ----------------------------------------
        with (
            tc.tile_pool(name="p1_sbuf", bufs=1) as p1,
            tc.tile_pool(name="p1_stage", bufs=6) as stage,
            tc.tile_pool(name="p1_psum", bufs=4, space="PSUM") as pp,
        ):
            xT_sb = p1.tile([P, HO, S], F32R)
            for o in range(HO):
                nc.sync.dma_start(
                    xT_sb[:, o, :],
                    xT.bitcast(F32R).rearrange("(o p) s -> p o s", p=P)[:, o, :],
                )
            w_sb = {}
            for name, wT in (("q", wqT), ("k", wkT), ("v", wvT)):
                w_sb[name] = p1.tile([P, HO, DH], F32R, name=f"w{name}_sb")
                for o in range(HO):
                    nc.sync.dma_start(
                        w_sb[name][:, o, :],
                        wT.bitcast(F32R).rearrange("(o p) d -> p o d", p=P)[:, o, :],
                    )

            # Q^T and K^T: psum[dq 128, s 512] = sum_o w[o, dq].T @ xT[o, s]
            for name, bias_sb, scale, tgt in (
                ("q", bq_sb, SCALE, qt_dram),
                ("k", bk_sb, 1.0, kt_dram),
            ):
                for t in range(NDQ):
                    for sc in range(NSQ):
                        ps = pp.tile([P, SQC], F32, name="proj_ps")
                        for o in range(HO):
                            nc.tensor.matmul(
                                ps[:],
                                w_sb[name][:, o, t * P : (t + 1) * P],
                                xT_sb[:, o, sc * SQC : (sc + 1) * SQC],
                                start=(o == 0),
                                stop=(o == HO - 1),
                            )
                        st = stage.tile([P, SQC], F32R, name="qk_stage")
                        nc.scalar.activation(
                            st[:], ps[:],
                            mybir.ActivationFunctionType.Identity,
                            bias=bias_sb[:, t : t + 1], scale=scale,
                        )
                        nc.sync.dma_start(tgt[t, :, sc * SQC : (sc + 1) * SQC], st[:])

            # V: psum[s 128, dv 512] = sum_o xT[o, s].T @ wv[o, dv]  (+ bv)
            for st_i in range(NSK):
                ps = pp.tile([P, DH], F32, name="v_ps")
                for o in range(HO):
                    nc.tensor.matmul(
                        ps[:],
                        xT_sb[:, o, st_i * P : (st_i + 1) * P],
                        w_sb["v"][:, o, :],
                        start=(o == 0),
                        stop=False,
                    )
                nc.tensor.matmul(  # bias row: ones[1,128].T @ bv[1,512]
                    ps[:], ind_sb[64:65, :], bv_sb[64:65, :],
                    start=False, stop=True, tile_position=(64, 0),
                )
                nc.vector.tensor_copy(v_sb[:, st_i, :], ps[:])

        # ---- Phase 2: attention ------------------------------------------
        with (
            tc.tile_pool(name="p2_qk", bufs=2) as p2qk,
            tc.tile_pool(name="p2_p", bufs=36) as p2p,
            tc.tile_pool(name="p2_rc", bufs=2) as p2rc,
            tc.tile_pool(name="p2_bc", bufs=2) as p2bc,
            tc.tile_pool(name="ps_s", bufs=3, space="PSUM") as ps_s,
            tc.tile_pool(name="ps_pv", bufs=2, space="PSUM") as ps_pv,
            tc.tile_pool(name="ps_den", bufs=2, space="PSUM") as ps_den,
            tc.tile_pool(name="ps_bc", bufs=1, space="PSUM") as ps_bc,
        ):
            pending = None  # deferred normalize: (pv_ps, den_ps, hp, i)
            for hp in range(NHP):
                qt_hp = p2qk.tile([P, S], F32R, name="qt_hp")
                nc.sync.dma_start(qt_hp[:], qt_dram[hp])
                kt_hp = p2qk.tile([P, S], F32R, name="kt_hp")
                nc.sync.dma_start(kt_hp[:], kt_dram[hp])

                for i in range(NSQ):
                    nj = 4 * i + 4  # causal s_k chunks
                    sq = slice(i * SQC, (i + 1) * SQC)
                    p_tiles = []
                    for j in range(nj):
                        sk = slice(j * P, (j + 1) * P)
                        pj = []
                        for h, (pb, tp) in enumerate(((0, (0, 0)), (64, (64, 0)))):
                            sc_ps = ps_s.tile([P, SQC], F32, name="sc_ps")
                            nc.tensor.matmul(
                                sc_ps[:],
                                kt_hp[pb : pb + 64, sk],
                                qt_hp[pb : pb + 64, sq],
                                start=True, stop=True,
                                tile_position=tp,
                            )
                            pt = p2p.tile([P, SQC], BF16, name="p_tile")
                            nc.scalar.activation(
                                pt[:], sc_ps[:], mybir.ActivationFunctionType.Exp
                            )
                            if j >= 4 * i:  # diagonal block: causal mask
                                nc.gpsimd.affine_select(
                                    pt[:], pt[:],
                                    pattern=[[1, SQC]],
                                    compare_op=mybir.AluOpType.is_ge,
                                    fill=0.0,
                                    base=SQC * i - P * j,
                                    channel_multiplier=-1,
                                )
                            pj.append(pt)
                        p_tiles.append(pj)

                    den_ps = ps_den.tile([P, SQC], F32, name="den_ps")
                    pv_ps = ps_pv.tile([P, SQC], F32, name="pv_ps")
                    for j in range(nj):
                        st, sp = (j == 0), (j == nj - 1)
                        for h, (rowbase, colpos) in enumerate(((0, 0), (32, 32))):
                            nc.tensor.matmul(
                                den_ps[rowbase : rowbase + 1, :],
                                ones_col[:, 0:1],
                                p_tiles[j][h][:],
                                start=st, stop=sp,
                                tile_position=(0, colpos),
                            )
                        for h in range(2):
                            dv = slice(hp * P + h * 64, hp * P + h * 64 + 64)
                            nc.tensor.matmul(
                                pv_ps[h * 64 : h * 64 + 64, :],
                                v_sb[:, j, dv],
                                p_tiles[j][h][:],
                                start=st, stop=sp,
                                tile_position=(0, h * 64),
                            )

                    rc = p2rc.tile([P, SQC], F32R, name="rc")
                    nc.vector.reciprocal(rc[0:1, :], den_ps[0:1, :])
                    nc.vector.reciprocal(rc[32:33, :], den_ps[32:33, :])

                    if pending is not None:
                        _flush_norm(nc, ps_bc, p2bc, ind_sb, outT_sb, *pending)
                    pending = (pv_ps, rc, hp, i)
            _flush_norm(nc, ps_bc, p2bc, ind_sb, outT_sb, *pending)

        # ---- Phase 3: output projection ----------------------------------
        with (
            tc.tile_pool(name="p3_wo", bufs=1) as p3w,
            tc.tile_pool(name="p3_stage", bufs=4) as p3s,
            tc.tile_pool(name="ps_o", bufs=4, space="PSUM") as ps_o,
        ):
            wo_sb = p3w.tile([P, NDQ, H], F32R)
            for o in range(NDQ):
                nc.sync.dma_start(
                    wo_sb[:, o, :],
                    woT.bitcast(F32R).rearrange("(o p) m -> p o m", p=P)[:, o, :],
                )
            for st_i in range(NSK):
                ss = slice(st_i * P, (st_i + 1) * P)
                for mc in range(2):
                    ms = slice(mc * SQC, (mc + 1) * SQC)
                    ps = ps_o.tile([P, SQC], F32, name="o_ps")
                    for ko in range(NDQ):
                        nc.tensor.matmul(
                            ps[:],
                            r(outT_sb[:, ko, ss]),
                            r(wo_sb[:, ko, ms]),
                            start=(ko == 0), stop=(ko == NDQ - 1),
                        )
                    ot = p3s.tile([P, SQC], F32, name="o_stage")
                    nc.vector.tensor_copy(ot[:], ps[:])
                    nc.sync.dma_start(out[ss, ms], ot[:])

    nc.compile()
    return nc


def _flush_norm(nc, ps_bc, bc_pool, ind_sb, outT_sb, pv_ps, rc, hp, i):
    """outT[:, hp, sq(i)] = pv_ps * broadcast(1/denom) via K=1 matmuls."""
    bc = ps_bc.tile([P, SQC], F32, name="bc_ps")
    nc.tensor.matmul(
        bc[:], ind_sb[0:1, :], rc[0:1, :],
        start=True, stop=False, tile_position=(0, 0),
    )
    nc.tensor.matmul(
        bc[:], ind_sb[32:33, :], rc[32:33, :],
        start=False, stop=True, tile_position=(32, 0),
    )
    bc_sb = bc_pool.tile([P, SQC], F32, name="bc_sb")
    nc.scalar.activation(bc_sb[:], bc[:], mybir.ActivationFunctionType.Copy)
    nc.vector.tensor_mul(
        outT_sb[:, hp, i * SQC : (i + 1) * SQC], pv_ps[:], bc_sb[:]
    )


_NC_CACHE = [None]
LAST_RESULT = [None]


def kernel(x, Wq, bq, Wk, bk, Wv, bv, Wo, bo):
    x, Wq, bq, Wk, bk, Wv, bv, Wo, bo = (
        np.asarray(a, dtype=np.float32) for a in (x, Wq, bq, Wk, bk, Wv, bv, Wo, bo)
    )
    if _NC_CACHE[0] is None:
        _NC_CACHE[0] = build_kernel()
    nc = _NC_CACHE[0]

    cmat = np.zeros((P, P), np.float32)
    cmat[0, 0:64] = 1.0
    cmat[32, 64:128] = 1.0
    cmat[64, :] = 1.0

    in_maps = []
    for c in range(8):
        b, g = c // 2, c % 2
        hs = slice(DH * g, DH * (g + 1))
        in_maps.append({
            "xT": np.ascontiguousarray(x[b].T),
            "wqT": np.ascontiguousarray(Wq[hs].T),
            "wkT": np.ascontiguousarray(Wk[hs].T),
            "wvT": np.ascontiguousarray(Wv[hs].T),
            "woT": np.ascontiguousarray(Wo[:, hs].T),
            "bq": np.ascontiguousarray(bq[hs]) * np.float32(SCALE),
            "bk": np.ascontiguousarray(bk[hs]),
            "bv": np.ascontiguousarray(bv[hs]),
            "cmat": cmat,
        })
    trace = bool(os.environ.get("BASS_PROFILE"))
    res = run_bass_kernel_spmd(
        nc, in_maps, core_ids=list(range(8)), trace=trace,
        tmpdir=os.environ.get("BASS_PROFILE_DIR") or None,
    )
    LAST_RESULT[0] = res
    out = np.empty((B, S, H), np.float32)
    for b in range(B):
        out[b] = res.results[2 * b]["out"] + res.results[2 * b + 1]["out"] + bo
    return out
